# revision 21
# baseline (speedup 1.0000x reference)
"""Trainium2 Bass kernel for DeformableSincConv1d (v8, padded polyphase).

Data parallel over batch: 4 rows/core on 8 cores. Per core, per batch-pair:
  1. im2col load (fp16) in PADDED POLYPHASE window order: window pi = 384*r+s
     (s < 320) holds x[10*l + j] for l = 10*s + r; 384 = 3 chunks per plane
     makes the dst AP rectangular so ONE 4-D dma_start covers 5 planes of a
     row (plus a small tail start).  Downstream X0P/dd are plane-major and
     every op (including the final-conv rhs) is contiguous.
  2. One PE transpose per 128-window chunk over cols 0..116 (row 0 of the
     pair at cols 0..52, row 1 at 64..116) -> psum [117, 128]; a single
     Act/DVE copy evacuates psum[0:115] straight into X0P.
  3. Sampling, partition shifts as matmuls: qp = wr2^T X0P (offset conv),
     ep = Emat^T X0P (= X0P[j+1]-X0P[j], col 50 zero), em = Mneg^T X0P
     (= X0P[j-1]-X0P[j], col 0 zero). Act computes o+ = relu(qp + b) and
     o- = relu(-qp - b); dd = X0P + o+*ep + o-*em reproduces the clipped
     lerp exactly (offsets |o| < 1).  One merged [115]-partition add writes
     both batch rows of dd.
  4. Final conv: stacked rotated-filter decomposition; per t0 two matmuls
     accumulate in psum (A-half from plane a, B-half from plane a+1, the
     a+1=10 wrap is plane 0 shifted one s).  Three t0 per 2-bank psum tile
     (51 = 3*17), full s range; the evacuation writes (s, t0)-triples so
     consecutive stores are 6-byte adjacent in fp16 ysb; y leaves as fp16.
"""

import sys

import numpy as np

if "/opt/trn_rl_repo" not in sys.path:
    sys.path.insert(0, "/opt/trn_rl_repo")

SR = 16000
C_OUT = 80
K = 51
STRIDE = 10
HALF = (K - 1) // 2

B_FULL = 32
N_CORES = 8
B_LOC = B_FULL // N_CORES
L_FULL = 32000

R1 = 64          # partition base of second row in pair tiles
NP = R1 + K      # 115 rows in pair tiles
KC = K * C_OUT


def _derive(L):
    L_out = (L - K) // STRIDE + 1
    T_out = (L_out * K - K) // STRIDE + 1
    XLEN = L + 48
    return L_out, T_out, XLEN


def _host_filters(hz, band):
    hzc = np.clip(hz.astype(np.float32), 0.0, SR / 2).astype(np.float32)
    bandc = np.clip(band.astype(np.float32), 3.0, SR / 2).astype(np.float32)
    t_right = (np.arange(1, HALF + 1, dtype=np.float32) / np.float32(SR)).astype(np.float32)
    low = (hzc - bandc / 2).astype(np.float32)
    high = (hzc + bandc / 2).astype(np.float32)

    def sinc(t):
        ts = np.where(t == 0, np.float32(1.0), t)
        return np.where(t == 0, np.float32(1.0), np.sin(ts) / ts).astype(np.float32)

    a1 = (2 * high).astype(np.float32)
    a2 = (2 * low).astype(np.float32)
    bp_left = (a1 * sinc(a1 * t_right) - a2 * sinc(a2 * t_right)).astype(np.float32)
    bp = np.concatenate([bp_left, np.ones((C_OUT, 1), np.float32), bp_left[:, ::-1]], axis=1)
    return (bp / (2 * bandc)).astype(np.float32)  # [C_OUT, K]


def _host_fstk(filt, L):
    """Stacked rotated filter matrix [NP, K*C_OUT]: rows 0..50 the A-half
    (taps landing on plane a), rows 64..114 the B-half (plane a+1); the rhs
    is the DDS tile whose rows 64.. hold dd shifted one plane."""
    L_out, T_out, _ = _derive(L)
    F = np.zeros((NP, K, C_OUT), np.float32)
    for t0 in range(K):
        a = (STRIDE * t0) // K
        ns = (T_out - 1 - t0) // K + 1
        for k2 in range(K):
            kstar = (k2 + STRIDE * t0) % K
            lstar = (STRIDE * t0 + k2) // K
            assert lstar in (a, a + 1)
            F[kstar + R1 * (lstar - a), t0, :] = filt[:, k2]
        assert a + 1 + STRIDE * (ns - 1) <= L_out - 1
    return F.reshape(NP, KC)


def _host_shift_mats():
    """emat: col j -> X0P[j+1]-X0P[j] (j<50), col 50 zero.
    mmatn: col j -> X0P[j-1]-X0P[j] (j>0), col 0 zero.  Block-diag at 0, R1."""
    emat = np.zeros((NP, NP), np.float32)
    mmatn = np.zeros((NP, NP), np.float32)
    for base in (0, R1):
        for j in range(K - 1):
            emat[base + j + 1, base + j] = 1.0
            emat[base + j, base + j] = -1.0
        for j in range(1, K):
            mmatn[base + j - 1, base + j] = 1.0
            mmatn[base + j, base + j] = -1.0
    return emat, mmatn


def build_program(B_loc=B_LOC, L=L_FULL, debug=False):
    import concourse.bacc as bacc
    import concourse.tile as tile
    from concourse import bass, mybir

    f32 = mybir.dt.float32
    f16 = mybir.dt.float16
    Act = mybir.ActivationFunctionType
    Alu = mybir.AluOpType

    L_out, T_out, XLEN = _derive(L)
    NSMAX = (T_out - 1) // K + 1    # 320 real s slots per plane
    SPL = 384                       # padded plane pitch (3 x 128 chunks)
    LSAMP = STRIDE * SPL            # padded polyphase extent (3840)
    NCHUNK = LSAMP // 128           # 30
    NCH = NCHUNK // 2               # chunks per xx half-tile (15 = 5 planes)
    CC = 480
    NCC = LSAMP // CC               # 8
    assert B_loc == 4

    nc = bacc.Bacc("TRN2", target_bir_lowering=False, debug=debug)

    x_d = nc.dram_tensor("x", [B_loc, XLEN], f16, kind="ExternalInput")
    wr2_d = nc.dram_tensor("wr2", [NP, NP], f16, kind="ExternalInput")
    emat_d = nc.dram_tensor("emat", [NP, NP], f16, kind="ExternalInput")
    mmatn_d = nc.dram_tensor("mmatn", [NP, NP], f16, kind="ExternalInput")
    offb2_d = nc.dram_tensor("offb2", [NP, 1], f32, kind="ExternalInput")
    negoffb2_d = nc.dram_tensor("negoffb2", [NP, 1], f32, kind="ExternalInput")
    fstk_d = nc.dram_tensor("fstk", [NP, KC], f16, kind="ExternalInput")
    ident_d = nc.dram_tensor("ident", [128, 128], f16, kind="ExternalInput")
    y_d = nc.dram_tensor("y", [B_loc, C_OUT, T_out], f16, kind="ExternalOutput")

    xap = x_d[:]

    with tile.TileContext(nc) as tc:
        with (
            tc.tile_pool(name="consts", bufs=1) as consts,
            tc.tile_pool(name="xxp", bufs=4) as xxp,
            tc.tile_pool(name="x0p", bufs=2) as x0p,
            tc.tile_pool(name="qsp", bufs=4) as qsp,
            tc.tile_pool(name="emp", bufs=6) as emp,
            tc.tile_pool(name="ddp", bufs=2) as ddp,
            tc.tile_pool(name="ddsp", bufs=3) as ddsp,
            tc.tile_pool(name="ysbp", bufs=2) as ysbp,
            tc.tile_pool(name="tpsum", bufs=1, space="PSUM") as tpsum,
            tc.tile_pool(name="qpsum", bufs=3, space="PSUM") as qpsum,
            tc.tile_pool(name="fpsum", bufs=4, space="PSUM") as fpsum,
        ):
            wr2_sb = consts.tile([NP, NP], f16)
            nc.sync.dma_start(out=wr2_sb[:], in_=wr2_d[:])
            emat_sb = consts.tile([NP, NP], f16)
            nc.sync.dma_start(out=emat_sb[:], in_=emat_d[:])
            mmatn_sb = consts.tile([NP, NP], f16)
            nc.sync.dma_start(out=mmatn_sb[:], in_=mmatn_d[:])
            offb2_sb = consts.tile([NP, 1], f32)
            nc.sync.dma_start(out=offb2_sb[:], in_=offb2_d[:])
            negoffb2_sb = consts.tile([NP, 1], f32)
            nc.sync.dma_start(out=negoffb2_sb[:], in_=negoffb2_d[:])
            fstk_sb = consts.tile([NP, KC], f16)
            nc.sync.dma_start(out=fstk_sb[:], in_=fstk_d[:])
            ident_sb = consts.tile([128, 128], f16)
            nc.sync.dma_start(out=ident_sb[:], in_=ident_d[:])

            def ecopy(eng, dst, src):
                if eng is nc.scalar:
                    eng.copy(dst, src)
                else:
                    eng.tensor_copy(dst, src)

            def load_half_dma(p, half, eng):
                """One xx half-tile = 5 planes (15 chunks).  Per (row, plane):
                one 3-D dma_start for s 0..255 and one for s 256..319; the
                triggers issue from `eng`'s queue so the four half-tiles post
                their descriptors in parallel."""
                xx = xxp.tile([128, NCH, 128], f16, tag="xx")
                nc.vector.memset(xx[:, :, 53:64], 0.0)
                nc.vector.memset(xx[64:128, 2:NCH:3, :], 0.0)
                r_lo = 5 * half
                for ri in range(2):
                    col0 = R1 * ri
                    for r in range(5):
                        row_off = (2 * p + ri) * XLEN + 10 * (r_lo + r)
                        c0 = 3 * r
                        eng.dma_start(
                            out=xx[:, c0:c0 + 2, col0:col0 + 53],
                            in_=bass.AP(tensor=xap.tensor, offset=row_off,
                                        ap=[[100, 128], [12800, 2], [1, 53]]))
                        eng.dma_start(
                            out=xx[0:64, c0 + 2, col0:col0 + 53],
                            in_=bass.AP(tensor=xap.tensor,
                                        offset=row_off + 25600,
                                        ap=[[100, 64], [1, 53]]))
                return xx

            def load_half_tp(p, half, xx, X0P):
                for g in range(2):
                    n = 8 if g == 0 else NCH - 8
                    pt = tpsum.tile([117, 1024], f16, tag="pt")
                    for c in range(n):
                        nc.tensor.transpose(pt[:, c * 128:(c + 1) * 128],
                                            xx[:, 8 * g + c, 0:117],
                                            ident_sb[:])
                    lo = NCH * 128 * half + g * 1024
                    nw = n * 128
                    ecopy([nc.vector, nc.scalar][(p + half + g) % 2],
                          X0P[:, lo:lo + nw], pt[0:NP, :nw])

            def alloc_pair():
                X0P = x0p.tile([NP, LSAMP], f16)
                dd = ddp.tile([NP, LSAMP], f16, tag="dd")
                return X0P, dd

            def front_chunk(state, c7):
                """Sampling chain for one 480-column chunk; contiguous ops."""
                X0P, dd = state
                sl = slice(c7 * CC, (c7 + 1) * CC)
                qp = qpsum.tile([NP, CC], f32, tag="qps")
                nc.tensor.matmul(qp[:], wr2_sb[:], X0P[:, sl],
                                 start=True, stop=True)
                ep = qpsum.tile([NP, CC], f32, tag="qps")
                nc.tensor.matmul(ep[:], emat_sb[:], X0P[:, sl],
                                 start=True, stop=True)
                em = qpsum.tile([NP, CC], f32, tag="qps")
                nc.tensor.matmul(em[:], mmatn_sb[:], X0P[:, sl],
                                 start=True, stop=True)
                QSP = qsp.tile([NP, CC], f16, tag="qs")
                nc.scalar.activation(QSP[:], qp[:], Act.Relu,
                                     bias=offb2_sb[:], scale=1.0)
                QSM = qsp.tile([NP, CC], f16, tag="qs")
                nc.vector.tensor_scalar(QSM[:], qp[:], offb2_sb[:], 0.0,
                                        op0=Alu.add, op1=Alu.min)
                mA = emp.tile([NP, CC], f16, tag="em")
                nc.vector.tensor_mul(mA[:], ep[:], QSP[:])
                mB = emp.tile([NP, CC], f16, tag="em")
                nc.vector.tensor_mul(mB[:], em[:], QSM[:])
                S = emp.tile([NP, CC], f16, tag="em")
                (nc.vector if c7 % 2 else nc.gpsimd).tensor_add(
                    S[:], mA[:], mB[:])
                nc.gpsimd.tensor_add(dd[:, sl], X0P[:, sl], S[:])

            SCATTER = {0: nc.scalar, 1: nc.vector}

            def build_dds(dd, base):
                """Per-row stacked rhs: rows 0..50 = dd row slice, rows
                64..114 = the same shifted one plane (a+1), with the plane-10
                wrap = plane 0 shifted one s.  Three big SBUF->SBUF copies,
                off the critical path."""
                DDS = ddsp.tile([NP, LSAMP], f16, tag="dds")
                nc.vector.memset(DDS[32:R1, :], 0.0)
                nc.sync.dma_start(out=DDS[0:K, :], in_=dd[base:base + K, :])
                nc.sync.dma_start(out=DDS[R1:NP, 0:9 * SPL],
                                  in_=dd[base:base + K, SPL:10 * SPL])
                nc.sync.dma_start(out=DDS[R1:NP, 9 * SPL:9 * SPL + NSMAX],
                                  in_=dd[base:base + K, 1:1 + NSMAX])
                return DDS

            def fmm(fp_slice, DDS, t0, s_lo, n):
                a = (STRIDE * t0) // K
                a0 = SPL * a + s_lo
                nc.tensor.matmul(fp_slice,
                                 fstk_sb[:, t0 * C_OUT:(t0 + 1) * C_OUT],
                                 DDS[:, a0:a0 + n],
                                 start=True, stop=True)

            HH = NSMAX // 2   # s-half size (160)

            def final_trip_h(r, DDS, ysb, t0, h):
                """t0, t0+1, t0+2 in one 1-bank psum tile per s-half; the
                evacuation writes (s, t0)-triples so consecutive stores are
                6-byte adjacent in ysb."""
                s_lo = h * HH
                ns = [(T_out - 1 - (t0 + i)) // K + 1 for i in range(3)]
                n = [min(x - s_lo, HH) for x in ns]
                fp = fpsum.tile([C_OUT, 3, HH], f32, tag="fp")
                for i in range(3):
                    fmm(fp[:, i, :n[i]], DDS, t0 + i, s_lo, n[i])
                base_t = t0 + K * s_lo
                n2 = n[2]
                yv3 = ysb[:, base_t:base_t + K * n2].rearrange(
                    "p (s q) -> p s q", q=K)[:, :, 0:3]
                sv = fp[:, :, :n2].rearrange("p t s -> p s t")
                ecopy(SCATTER[(r + t0 // 3 + h) % 2], yv3, sv)
                if n[1] > n2:
                    yt = ysb[:, base_t + K * n2:base_t + K * n2 + 2]
                    ecopy(SCATTER[(r + t0 // 3 + h + 1) % 2], yt,
                          fp[:, 0:2, n2])
                if n[0] > n[1]:
                    yt = ysb[:, base_t + K * n[1]:base_t + K * n[1] + 1]
                    ecopy(SCATTER[(r + t0 // 3 + h) % 2], yt,
                          fp[:, 0, n[1]:n[0]])

            def final_rowpass(r, DDS, ysb, front=None):
                """front: optional list of thunks interleaved between t0
                triples (pair-1 sampling chunks issued during row-0 final)."""
                i = 0
                for g in range(K // 3):
                    for h in range(2):
                        final_trip_h(r, DDS, ysb, 3 * g, h)
                    if front is not None and g % 2 == 0 and i < len(front):
                        front[i]()
                        i += 1
                if front is not None:
                    while i < len(front):
                        front[i]()
                        i += 1

            YSBW = T_out + 3 * K

            st0 = alloc_pair()
            st1 = alloc_pair()
            xx00 = load_half_dma(0, 0, nc.sync)
            xx01 = load_half_dma(0, 1, nc.sync)
            xx10 = load_half_dma(1, 0, nc.gpsimd)
            xx11 = load_half_dma(1, 1, nc.gpsimd)
            load_half_tp(0, 0, xx00, st0[0])
            for c7 in range(NCC // 2):
                front_chunk(st0, c7)
            load_half_tp(0, 1, xx01, st0[0])
            for c7 in range(NCC // 2, NCC):
                front_chunk(st0, c7)
            dds0 = build_dds(st0[1], 0)
            dds1 = build_dds(st0[1], R1)
            load_half_tp(1, 0, xx10, st1[0])
            load_half_tp(1, 1, xx11, st1[0])

            ysb0 = ysbp.tile([C_OUT, YSBW], f16, tag="ysb")
            front1 = [
                (lambda i=i: front_chunk(st1, i)) for i in range(NCC)
            ] + [lambda: build_dds2(), lambda: build_dds3()]
            dds23 = []

            def build_dds2():
                dds23.append(build_dds(st1[1], 0))

            def build_dds3():
                dds23.append(build_dds(st1[1], R1))

            final_rowpass(0, dds0, ysb0, front=front1)
            nc.sync.dma_start(out=y_d[0], in_=ysb0[:, :T_out])

            dds2, dds3 = dds23
            ysb1 = ysbp.tile([C_OUT, YSBW], f16, tag="ysb")
            final_rowpass(1, dds1, ysb1)
            nc.sync.dma_start(out=y_d[1], in_=ysb1[:, :T_out])

            ysb2 = ysbp.tile([C_OUT, YSBW], f16, tag="ysb")
            final_rowpass(2, dds2, ysb2)
            nc.sync.dma_start(out=y_d[2], in_=ysb2[:, :T_out])

            ysb3 = ysbp.tile([C_OUT, YSBW], f16, tag="ysb")
            final_rowpass(3, dds3, ysb3)
            nc.sync.dma_start(out=y_d[3], in_=ysb3[:, :T_out])

    nc.compile()
    return nc


def _host_inputs(x, hz, band, offset_w, offset_b, B_loc, L):
    """Build the per-core input maps."""
    L_out, T_out, XLEN = _derive(L)
    filt = _host_filters(hz, band)
    fstk = _host_fstk(filt, L).astype(np.float16)
    wr = offset_w[:, 0, :].T.astype(np.float32)  # [k_in, k_out]
    wr2 = np.zeros((NP, NP), np.float32)
    wr2[0:K, 0:K] = wr
    wr2[R1:NP, R1:NP] = wr
    emat, mmatn = _host_shift_mats()
    offb2 = np.zeros((NP, 1), np.float32)
    offb2[0:K, 0] = offset_b.astype(np.float32)
    offb2[R1:NP, 0] = offset_b.astype(np.float32)
    negoffb2 = -offb2
    ident = np.eye(128, dtype=np.float16)

    B = x.shape[0]
    xpad = np.zeros((B, XLEN), np.float16)
    xpad[:, 0:L] = x.astype(np.float16)

    n_cores = B // B_loc
    in_maps = []
    for i in range(n_cores):
        in_maps.append({
            "x": np.ascontiguousarray(xpad[i * B_loc:(i + 1) * B_loc]),
            "wr2": wr2.astype(np.float16),
            "emat": emat.astype(np.float16),
            "mmatn": (-mmatn).astype(np.float16),
            "offb2": offb2,
            "negoffb2": negoffb2,
            "fstk": fstk,
            "ident": ident,
        })
    return in_maps


_CACHED = {}


def _get_program():
    key = (B_LOC, L_FULL)
    if key not in _CACHED:
        _CACHED[key] = build_program(B_LOC, L_FULL)
    return _CACHED[key]


def kernel(x, hz, band, offset_w, offset_b):
    from concourse.bass_utils import run_bass_kernel_spmd

    x = np.asarray(x, dtype=np.float32)
    hz = np.asarray(hz, dtype=np.float32)
    band = np.asarray(band, dtype=np.float32)
    offset_w = np.asarray(offset_w, dtype=np.float32)
    offset_b = np.asarray(offset_b, dtype=np.float32)

    nc = _get_program()
    in_maps = _host_inputs(x, hz, band, offset_w, offset_b, B_LOC, L_FULL)
    res = run_bass_kernel_spmd(nc, in_maps, list(range(N_CORES)))
    outs = [res.results[i]["y"] for i in range(N_CORES)]
    return np.concatenate(outs, axis=0).astype(np.float32)
